# revision 2
# baseline (speedup 1.0000x reference)
"""MoE layer (B=4,S=1024,D=1024,F=4096,E=8,top-2) on 8 trn2 NeuronCores.

Strategy: expert-parallel sparse dispatch. The dense reference multiplies
every expert's FFN output by a combine weight that is zero for all but the
top-2 experts per token, so only the top-2 expert rows need computing.
Host computes gating (tiny: 4096x1024x8 matmul), builds per-expert token
lists, and each core runs one expert's FFN over just its tokens:

    y = combine_w * (gelu(x @ w1 + b1) @ w2 + b2)

Device kernel per core (C = padded token capacity):
  - xT [D, C] resident in SBUF (8 chunks of [128, C])
  - stream W1/W2 in 512-wide f-groups, ONCE (32 MB total per core)
  - mm1: psum_h[128f, tb] = sum_d w1T_chunk.T @ xT_chunk   (8 matmuls)
  - gelu(+b1) on ACT into H [128f, C] (rounds to fp32r when enabled)
  - mm2: psum_y[128tok, 512d] += H_chunk.T @ w2_chunk      (4 matmuls)
    plus a K=1 matmul adding b2 broadcast on the first group
  - spill-add psum_y into SBUF Y accumulator (DVE), 8 groups total
  - final per-token scale by combine weight, DMA out

Host scatters the 8 per-expert outputs back (each token appears in
exactly 2 expert lists; within one list tokens are unique, so fancy
indexed += per expert is exact).
"""

import os
import numpy as np

B, S, D, F, E, TOPK = 4, 1024, 1024, 4096, 8, 2
NCORES = 8
FG = 512                      # f-group width streamed per step
NFC = FG // 128               # f-chunks (128-wide) per group
NG = F // FG                  # number of f-groups
CMAX = 1280                   # max tokens per core per round (SBUF limit)

# f32r: PE runs fp32 data at full (bf16) rate with ~1e-4 rounding.
# f32: exact IEEE fp32 matmuls at 1/4 rate.
USE_F32R = os.environ.get("KERNEL_DT", "f32r") == "f32r"

TRACE = False                 # test.py sets this for profiling runs
LAST_EXEC_TIME_NS = None
LAST_RESULTS = None

_PROGRAM_CACHE = {}


def _split_blocks(ncols):
    """Split ncols (multiple of 128) into matmul moving-dim blocks.

    Blocks are <=512 and, when possible, >=256 (fp32r runs full rate
    only for moving dim >= 256)."""
    n = ncols // 128
    out = []
    while n > 0:
        if n % 4 == 1 and n >= 3:
            take = 3
        elif n >= 4:
            take = 4
        else:
            take = n
        out.append(take * 128)
        n -= take
    return out


def _build_program(C):
    import concourse.bacc as bacc
    import concourse.tile as tile
    from concourse import mybir

    f32 = mybir.dt.float32
    mmdt = mybir.dt.float32r if USE_F32R else f32

    NT = C // 128
    blocks = _split_blocks(C)

    nc = bacc.Bacc(None, target_bir_lowering=False)
    xt = nc.dram_tensor("xt", [D, C], f32, kind="ExternalInput")
    w1 = nc.dram_tensor("w1", [D, F], f32, kind="ExternalInput")
    b1c = nc.dram_tensor("b1c", [128, F // 128], f32, kind="ExternalInput")
    w2 = nc.dram_tensor("w2", [F, D], f32, kind="ExternalInput")
    b2 = nc.dram_tensor("b2", [1, D], f32, kind="ExternalInput")
    ones = nc.dram_tensor("ones", [1, 128], f32, kind="ExternalInput")
    scale = nc.dram_tensor("scale", [128, NT], f32, kind="ExternalInput")
    y = nc.dram_tensor("y", [C, D], f32, kind="ExternalOutput")

    # casting DMA (fp32 -> fp32r rounding) must go through gpsimd (SWDGE)
    cast_eng = nc.gpsimd if USE_F32R else nc.sync

    with tile.TileContext(nc) as tc:
        with (
            tc.tile_pool(name="resid", bufs=1) as resid,
            tc.tile_pool(name="w1p", bufs=2) as w1p,
            tc.tile_pool(name="w2p", bufs=2) as w2p,
            tc.tile_pool(name="hp", bufs=2) as hp,
            tc.tile_pool(name="yop", bufs=2) as yop,
            tc.tile_pool(name="ph", bufs=3, space="PSUM") as php,
            tc.tile_pool(name="py", bufs=4, space="PSUM") as pyp,
        ):
            # ---- resident loads ----
            xt_sb = []
            for c in range(8):
                t = resid.tile([128, C], mmdt, tag=f"xt{c}", name=f"xt{c}")
                cast_eng.dma_start(t[:], xt[c * 128:(c + 1) * 128, :])
                xt_sb.append(t)
            b1_sb = resid.tile([128, F // 128], f32, tag="b1")
            nc.sync.dma_start(b1_sb[:], b1c[:])
            b2_sb = resid.tile([1, D], mmdt, tag="b2")
            cast_eng.dma_start(b2_sb[:], b2[:])
            ones_sb = resid.tile([1, 128], mmdt, tag="ones")
            cast_eng.dma_start(ones_sb[:], ones[:])
            scale_sb = resid.tile([128, NT], f32, tag="scale")
            nc.sync.dma_start(scale_sb[:], scale[:])
            y_sb = [resid.tile([128, D], f32, tag=f"y{t}", name=f"ysb{t}")
                    for t in range(NT)]

            # ---- f-group loop ----
            for g in range(NG):
                # stream this group's weights
                w1g = []
                for c in range(8):
                    t = w1p.tile([128, FG], mmdt, tag=f"w1c{c}", name=f"w1g{g}c{c}")
                    cast_eng.dma_start(t[:], w1[c * 128:(c + 1) * 128,
                                               g * FG:(g + 1) * FG])
                    w1g.append(t)
                w2g = []
                for j in range(NFC):
                    t = w2p.tile([128, D], mmdt, tag=f"w2c{j}", name=f"w2g{g}c{j}")
                    fr = g * FG + j * 128
                    cast_eng.dma_start(t[:], w2[fr:fr + 128, :])
                    w2g.append(t)

                # mm1 + gelu -> H
                hg = []
                for j in range(NFC):
                    h = hp.tile([128, C], mmdt, tag=f"h{j}", name=f"hg{g}c{j}")
                    col = 0
                    for bw in blocks:
                        ph = php.tile([128, bw], f32, tag="ph", name=f"ph{g}_{j}_{col}")
                        for c in range(8):
                            nc.tensor.matmul(
                                ph[:],
                                w1g[c][:, j * 128:(j + 1) * 128],
                                xt_sb[c][:, col:col + bw],
                                start=(c == 0), stop=(c == 7))
                        fi = g * NFC + j
                        nc.scalar.activation(
                            h[:, col:col + bw], ph[:],
                            mybir.ActivationFunctionType.Gelu,
                            bias=b1_sb[:, fi:fi + 1])
                        col += bw
                    hg.append(h)

                # mm2 -> psum_y, spill-add into Y
                for t in range(NT):
                    for dh in range(2):
                        py = pyp.tile([128, 512], f32, tag="py", name=f"py{g}_{t}_{dh}")
                        for j in range(NFC):
                            nc.tensor.matmul(
                                py[:],
                                hg[j][:, t * 128:(t + 1) * 128],
                                w2g[j][:, dh * 512:(dh + 1) * 512],
                                start=(j == 0),
                                stop=(j == NFC - 1 and g != 0))
                        if g == 0:
                            # fold b2 in via a K=1 matmul (broadcast add)
                            nc.tensor.matmul(
                                py[:], ones_sb[:],
                                b2_sb[:, dh * 512:(dh + 1) * 512],
                                start=False, stop=True)
                            nc.vector.tensor_copy(
                                y_sb[t][:, dh * 512:(dh + 1) * 512], py[:])
                        else:
                            ysl = y_sb[t][:, dh * 512:(dh + 1) * 512]
                            nc.vector.tensor_tensor(
                                ysl, ysl, py[:], op=mybir.AluOpType.add)
                    if g == NG - 1:
                        yo = yop.tile([128, D], f32, tag="yo", name=f"yo{t}")
                        nc.vector.tensor_scalar_mul(
                            yo[:], y_sb[t][:], scale_sb[:, t:t + 1])
                        nc.sync.dma_start(y[t * 128:(t + 1) * 128, :], yo[:])

    nc.compile()
    return nc


def _get_program(C):
    if C not in _PROGRAM_CACHE:
        _PROGRAM_CACHE[C] = _build_program(C)
    return _PROGRAM_CACHE[C]


def kernel(x, gate_w, gate_b, w1, b1, w2, b2):
    global LAST_EXEC_TIME_NS, LAST_RESULTS
    from concourse.bass_utils import run_bass_kernel_spmd

    x = np.asarray(x, dtype=np.float32)
    gate_w = np.asarray(gate_w, dtype=np.float32)
    gate_b = np.asarray(gate_b, dtype=np.float32)
    w1 = np.asarray(w1, dtype=np.float32)
    b1 = np.asarray(b1, dtype=np.float32)
    w2 = np.asarray(w2, dtype=np.float32)
    b2 = np.asarray(b2, dtype=np.float32)

    N = B * S
    xf = np.ascontiguousarray(x.reshape(N, D))

    # ---- gating on host (fp32, matches jax.lax.top_k tie-breaking) ----
    logits = xf @ gate_w + gate_b                       # [N, E] fp32
    top2 = np.argsort(-logits, axis=1, kind="stable")[:, :TOPK]   # [N, 2]
    tl = np.take_along_axis(logits, top2, axis=1)
    tl64 = tl.astype(np.float64)
    ex = np.exp(tl64 - tl64.max(axis=1, keepdims=True))
    probs = (ex / ex.sum(axis=1, keepdims=True)).astype(np.float32)

    # aux loss (returned alongside output, like the reference)
    l64 = logits.astype(np.float64)
    sm = np.exp(l64 - l64.max(axis=1, keepdims=True))
    sm /= sm.sum(axis=1, keepdims=True)
    usage = sm.mean(axis=0)
    aux_loss = np.float32(E * np.sum(usage ** 2))

    # ---- per-expert token lists ----
    tok_lists, wt_lists = [], []
    for e in range(E):
        m = top2 == e                                    # [N, 2]
        sel = m.any(axis=1)
        toks = np.flatnonzero(sel)
        wts = probs[toks][m[toks]].astype(np.float32)    # one weight per token
        tok_lists.append(toks)
        wt_lists.append(wts)

    # split oversize expert loads into rounds of <= CMAX tokens
    nrounds = max(1, max((len(t) + CMAX - 1) // CMAX for t in tok_lists))
    seg = [[t[r * CMAX:(r + 1) * CMAX] for r in range(nrounds)] for t in tok_lists]
    wseg = [[w[r * CMAX:(r + 1) * CMAX] for r in range(nrounds)] for w in wt_lists]
    C = max(len(s) for ss in seg for s in ss)
    C = max(128, ((C + 127) // 128) * 128)

    nc = _get_program(C)
    NT = C // 128
    onesr = np.ones((1, 128), np.float32)
    w1e = [np.ascontiguousarray(w1[e]) for e in range(E)]
    w2e = [np.ascontiguousarray(w2[e]) for e in range(E)]
    b1e = [np.ascontiguousarray(b1[e].reshape(F // 128, 128).T) for e in range(E)]
    b2e = [np.ascontiguousarray(b2[e].reshape(1, D)) for e in range(E)]

    out = np.zeros((N, D), dtype=np.float32)
    LAST_EXEC_TIME_NS = None
    for r in range(nrounds):
        in_maps = []
        for e in range(E):
            toks, wts = seg[e][r], wseg[e][r]
            n = len(toks)
            xg = np.zeros((C, D), np.float32)
            if n:
                xg[:n] = xf[toks]
            sc = np.zeros(C, np.float32)
            sc[:n] = wts
            in_maps.append({
                "xt": np.ascontiguousarray(xg.T),
                "w1": w1e[e],
                "b1c": b1e[e],
                "w2": w2e[e],
                "b2": b2e[e],
                "ones": onesr,
                "scale": np.ascontiguousarray(sc.reshape(NT, 128).T),
            })
        res = run_bass_kernel_spmd(nc, in_maps, list(range(NCORES)), trace=TRACE)
        if res.exec_time_ns is not None:
            LAST_EXEC_TIME_NS = res.exec_time_ns
        LAST_RESULTS = res
        for e in range(E):
            toks = seg[e][r]
            if len(toks):
                out[toks] += res.results[e]["y"][:len(toks)]

    return out.reshape(B, S, D), aux_loss
